# revision 1
# baseline (speedup 1.0000x reference)
"""Trainium2 Bass kernel for an 8-layer dense transformer (B=4,T=1024,C=1024,
H=16,HS=64,V=32000), sharded over 8 NeuronCores as DP=4 (batch) x TP=2
(heads/FFN-hidden/vocab), with pairwise AllReduce after the attention
projection and FFN down-projection.

v2: bf16 matmul datapath (f32 PSUM accumulation, f32r residual/LN stats),
host-fused per-head+mix projection (PhW@Pm), layer-resident weights with a
handful of large contiguous DMAs per layer, finer causal-block skipping,
per-token-half tile splitting so AllReduces overlap compute, bf16 logits
(host upcast).
"""
import numpy as np
import ml_dtypes

import concourse.bass as bass
import concourse.mybir as mybir
import concourse.tile as tile
from concourse import bacc
from concourse.masks import make_identity

V, B, T, C, H, L = 32000, 4, 1024, 1024, 16, 8
HS = C // H          # 64
P = 128
NHC = H // 2         # heads per core = 8
NPR = NHC // 2       # head pairs = 4
CK = C // P          # channel chunks = 8
F = 4 * C
FS = F // 2          # ffn shard = 2048
FCK = FS // P        # f chunks = 16
VS = V // 2          # vocab shard = 16000
VT = 500             # vocab tile
NVT = VS // VT       # 32
TH = 512             # token half
GPT = T // P         # token groups = 8
SCALE = float(C) ** -0.5
EPS = 1e-5

F32 = mybir.dt.float32
F32R = mybir.dt.float32r
BF16 = mybir.dt.bfloat16
F16 = mybir.dt.float16
I32 = mybir.dt.int32
AF = mybir.ActivationFunctionType
ALU = mybir.AluOpType
GROUPS = [[0, 1], [2, 3], [4, 5], [6, 7]]
BF = ml_dtypes.bfloat16

_CACHE = {}


# ----------------------------------------------------------------------------
# host-side shard prep
# ----------------------------------------------------------------------------

def _prep_core(inp, core):
    b, tp = core // 2, core % 2
    heads = slice(tp * NHC, (tp + 1) * NHC)
    f32 = lambda a: np.ascontiguousarray(np.asarray(a, np.float32))
    bf = lambda a: np.ascontiguousarray(np.asarray(a).astype(BF))
    d = {}
    d["idx"] = np.ascontiguousarray(np.asarray(inp["idx"][b])
                                    .astype(np.int32).reshape(T, 1))
    d["tok_emb"] = bf(inp["tok_emb"])
    d["pos_emb"] = bf(inp["pos_emb"])
    for nm in ("Wq", "Wk"):
        w = np.asarray(inp[nm], np.float32)[:, heads]          # [L,8,C,HS]
        w = np.transpose(w, (0, 2, 1, 3)).reshape(L, CK, P, NPR, P)
        # [L, NPR, P(c within chunk), CK*P(f)] so per-pr DMA is contiguous
        w = np.transpose(w, (0, 3, 2, 1, 4)).reshape(L, NPR, P, CK * P)
        d[nm.lower()] = bf(w)
    wv = np.asarray(inp["Wv"], np.float32)[:, heads]           # [L,8,C,HS]
    wv = np.transpose(wv, (0, 2, 1, 3)).reshape(L, CK, P, NHC * HS)
    d["wv"] = bf(np.transpose(wv, (0, 2, 1, 3)).reshape(L, P, CK * NHC * HS))
    # fused per-head proj + mix: phpm[l] = PhW_flat[l] @ Pm[l]  [512, C]
    phw = np.asarray(inp["Ph_w"], np.float32)[:, heads].reshape(L, NHC * HS, C)
    pm = np.asarray(inp["Pm_w"], np.float32)
    phpm = np.einsum("loc,lcd->lod", phw, pm)                  # [L, 512, C]
    phpm = phpm.reshape(L, NPR, P, C).transpose(0, 2, 1, 3)    # [L,P,NPR,C]
    d["phpm"] = bf(phpm.reshape(L, P, NPR * C))
    w1 = np.asarray(inp["W1"], np.float32)[:, :, tp * FS:(tp + 1) * FS]
    w1 = w1.reshape(L, CK, P, FS).transpose(0, 2, 1, 3)
    d["w1"] = bf(w1.reshape(L, P, CK * FS))
    w2 = np.asarray(inp["W2"], np.float32)[:, tp * FS:(tp + 1) * FS]
    # [L, CK(ct), P(f within chunk), FCK*P] so per-ct DMA is contiguous
    w2 = w2.reshape(L, FCK, P, CK, P).transpose(0, 3, 2, 1, 4)
    d["w2"] = bf(w2.reshape(L, CK, P, FCK * P))
    d["b1s"] = np.ascontiguousarray(
        np.asarray(inp["b1"], np.float32)[:, tp * FS:(tp + 1) * FS]
        .reshape(L, FCK, P, 1))
    phb = np.asarray(inp["Ph_b"], np.float32)[:, heads].sum(1)   # [L, C]
    ab = np.einsum("lc,lcd->ld", phb, pm)
    if tp == 0:
        ab = ab + np.asarray(inp["Pm_b"], np.float32)
    d["attn_bias"] = np.ascontiguousarray(ab.reshape(L, CK, P, 1))
    b2 = (np.asarray(inp["b2"], np.float32) if tp == 0
          else np.zeros((L, C), np.float32))
    d["b2g"] = np.ascontiguousarray(b2.reshape(L, CK, P, 1))
    for nm, key in (("ln1_g", "ln1g"), ("ln1_b", "ln1b"),
                    ("ln2_g", "ln2g"), ("ln2_b", "ln2b")):
        d[key] = f32(inp[nm]).reshape(L, CK, P, 1)
    d["lnfg"] = f32(inp["lnf_g"]).reshape(CK, P, 1)
    d["lnfb"] = f32(inp["lnf_b"]).reshape(CK, P, 1)
    d["headw"] = bf(np.asarray(inp["head_w"], np.float32)
                    [:, tp * VS:(tp + 1) * VS].reshape(CK, P, VS))
    d["headb"] = f32(np.asarray(inp["head_b"], np.float32)
                     [tp * VS:(tp + 1) * VS].reshape(1, VS))
    return d


# ----------------------------------------------------------------------------
# device program
# ----------------------------------------------------------------------------

def build_nc(taps=False):
    nc = bacc.Bacc("TRN2", target_bir_lowering=False, debug=False, num_devices=8)

    def din(name, shape, dt_):
        return nc.dram_tensor(name, list(shape), dt_, kind="ExternalInput")

    idx_d = din("idx", [T, 1], I32)
    tok_d = din("tok_emb", [V, C], BF16)
    pos_d = din("pos_emb", [T, C], BF16)
    wq_d = din("wq", [L, NPR, P, CK * P], BF16)
    wk_d = din("wk", [L, NPR, P, CK * P], BF16)
    wv_d = din("wv", [L, P, CK * NHC * HS], BF16)
    phpm_d = din("phpm", [L, P, NPR * C], BF16)
    w1_d = din("w1", [L, P, CK * FS], BF16)
    w2_d = din("w2", [L, CK, P, FCK * P], BF16)
    b1s_d = din("b1s", [L, FCK, P, 1], F32)
    ab_d = din("attn_bias", [L, CK, P, 1], F32)
    b2g_d = din("b2g", [L, CK, P, 1], F32)
    ln1g_d = din("ln1g", [L, CK, P, 1], F32)
    ln1b_d = din("ln1b", [L, CK, P, 1], F32)
    ln2g_d = din("ln2g", [L, CK, P, 1], F32)
    ln2b_d = din("ln2b", [L, CK, P, 1], F32)
    lnfg_d = din("lnfg", [CK, P, 1], F32)
    lnfb_d = din("lnfb", [CK, P, 1], F32)
    hw_d = din("headw", [CK, P, VS], BF16)
    hb_d = din("headb", [1, VS], F32)

    logits_d = nc.dram_tensor("logits", [T, VS], BF16, kind="ExternalOutput")

    tap_d = {}
    if taps:
        for nm in (["t_embed", "t_xn1", "t_q0", "t_k0", "t_v0", "t_ew", "t_ot",
                    "t_sa", "t_x1", "t_xn2", "t_h", "t_ffn", "t_x2",
                    "t_xf", "t_lg", "t_den", "t_rcp"]
                   + [f"t_xl{i}" for i in range(L)]):
            tap_d[nm] = nc.dram_tensor(nm, [P, 16], F32, kind="ExternalOutput")

    from contextlib import ExitStack
    with tile.TileContext(nc) as tc, ExitStack() as st:
        # ------- static pools
        cst = st.enter_context(tc.tile_pool(name="cst", bufs=1))
        xtp = st.enter_context(tc.tile_pool(name="xtp", bufs=1))
        wst = st.enter_context(tc.tile_pool(name="wst", bufs=2))
        wvp = st.enter_context(tc.tile_pool(name="wvp", bufs=1))
        php = st.enter_context(tc.tile_pool(name="php", bufs=1))
        w1p = st.enter_context(tc.tile_pool(name="w1p", bufs=1))
        w2p = st.enter_context(tc.tile_pool(name="w2p", bufs=4))
        lnp = st.enter_context(tc.tile_pool(name="lnp", bufs=2))
        tpp = st.enter_context(tc.tile_pool(name="tpp", bufs=2))
        lnw = st.enter_context(tc.tile_pool(name="lnw", bufs=1))
        lnwp = st.enter_context(tc.tile_pool(name="lnwp", bufs=1,
                                             space="PSUM"))
        sap = st.enter_context(tc.tile_pool(name="sap", bufs=2))
        fcp = sap
        drp = st.enter_context(tc.tile_pool(name="drp", bufs=4, space="DRAM"))

        # residual stream, split by token half: [p, k, t-local]
        xTh = [xtp.tile([P, CK, TH], F32R, name=f"xT{i}") for i in range(2)]

        ones_f = cst.tile([P, P], F32)
        nc.vector.memset(ones_f[:], 1.0)
        ones_r = cst.tile([P, P], F32R)
        nc.vector.tensor_copy(ones_r[:], ones_f[:])
        ident = cst.tile([P, P], F32)
        make_identity(nc, ident[:])
        ident_b = cst.tile([P, P], BF16)
        nc.vector.tensor_copy(ident_b[:], ident[:])
        eps_b = cst.tile([P, 1], F32)
        nc.vector.memset(eps_b[:], EPS)
        m0 = cst.tile([P, P], F32)            # keep t(free) >= u(part)
        nc.gpsimd.memset(m0[:], 0.0)
        nc.gpsimd.affine_select(
            out=m0[:], in_=m0[:], compare_op=ALU.is_ge,
            fill=-1e9, base=0, pattern=[[1, P]], channel_multiplier=-1)

        def tap(nm, src_ap):
            if not taps:
                return
            n = src_ap.free_size()
            p = src_ap.shape[0]
            if src_ap.dtype != F32:
                tmp = tpp.tile([P, 16], F32, name="tapt")
                nc.vector.tensor_copy(tmp[:p, :n], src_ap)
                src_ap = tmp[:p, :n]
            nc.sync.dma_start(tap_d[nm].ap()[:p, :n], src_ap)

        # ------- embedding: gather + pos, transpose into xTh
        with (tc.tile_pool(name="emb", bufs=3) as emb,
              tc.tile_pool(name="embp", bufs=4, space="PSUM") as embp):
            idx_sb = emb.tile([P, GPT], I32, name="idx_sb")
            nc.sync.dma_start(idx_sb[:],
                              idx_d.ap().rearrange("(g p) o -> p (g o)", p=P))
            for g in range(GPT):
                th, lg = g // 4, (g % 4) * P
                ge = emb.tile([P, C], BF16, name="ge")
                nc.gpsimd.indirect_dma_start(
                    out=ge[:], out_offset=None, in_=tok_d.ap(),
                    in_offset=bass.IndirectOffsetOnAxis(ap=idx_sb[:, g:g + 1],
                                                        axis=0))
                pe = emb.tile([P, C], BF16, name="pe")
                nc.sync.dma_start(pe[:], pos_d.ap()[g * P:(g + 1) * P, :])
                nc.vector.tensor_add(ge[:], ge[:], pe[:])
                for k in range(CK):
                    pt = embp.tile([P, P], BF16, name="pt")
                    nc.tensor.transpose(pt[:], ge[:, k * P:(k + 1) * P],
                                        ident_b[:])
                    nc.scalar.activation(xTh[th][:, k, lg:lg + P], pt[:],
                                         AF.Copy)
        tap("t_embed", xTh[0][:, 0, :16].bitcast(F32))

        # ------- layernorm of one token-half into dst[:, :, dst_ts] (bf16)
        def ln_half(dst, dst_ts, src, g_sb, b_sb, sbp, psp):
            sx = psp.tile([1, TH], F32, name="sx")
            sq = psp.tile([1, TH], F32, name="sq")
            for k in range(CK):
                sqk = sbp.tile([P, TH], F32R, name="sqk")
                nc.vector.tensor_mul(sqk[:], src[:, k, :], src[:, k, :])
                nc.tensor.matmul(sx[:], ones_r[:, :1], src[:, k, :],
                                 start=(k == 0), stop=(k == CK - 1))
                nc.tensor.matmul(sq[:], ones_r[:, :1], sqk[:],
                                 start=(k == 0), stop=(k == CK - 1))
            mean = sbp.tile([1, TH], F32, name="mean")
            nc.vector.tensor_scalar_mul(mean[:], sx[0:1], 1.0 / C)
            msq = sbp.tile([1, TH], F32, name="msq")
            nc.vector.tensor_scalar_mul(msq[:], sq[0:1], 1.0 / C)
            m2 = sbp.tile([1, TH], F32, name="m2")
            nc.vector.tensor_mul(m2[:], mean[:], mean[:])
            var = sbp.tile([1, TH], F32, name="var")
            nc.vector.tensor_sub(var[:], msq[:], m2[:])
            std = sbp.tile([1, TH], F32, name="std")
            nc.scalar.activation(std[:], var[:], AF.Sqrt,
                                 bias=eps_b[0:1, :1])
            rstd = sbp.tile([1, TH], F32, name="rstd")
            nc.vector.reciprocal_approx_fast(rstd[:], std[:])
            bmean_s = sbp.tile([P, TH], F32, name="bmean_s")
            nc.gpsimd.partition_broadcast(bmean_s[:], mean[:])
            brstd_s = sbp.tile([P, TH], F32, name="brstd_s")
            nc.gpsimd.partition_broadcast(brstd_s[:], rstd[:])
            for k in range(CK):
                t1 = sbp.tile([P, TH], F32R, name="t1")
                nc.vector.tensor_sub(t1[:], src[:, k, :], bmean_s[:])
                nc.vector.tensor_mul(t1[:], t1[:], brstd_s[:])
                with nc.allow_low_precision(reason="bf16 ln out"):
                    nc.vector.tensor_scalar(dst[:, k, dst_ts], t1[:],
                                            g_sb[:, k:k + 1], b_sb[:, k:k + 1],
                                            ALU.mult, ALU.add)

        def load_vec(pool, dram, l, n, name):
            t = pool.tile([P, n], F32, name=name)
            src = dram.ap()[l] if l is not None else dram.ap()
            nc.sync.dma_start(t[:], src.rearrange("k p o -> p (k o)"))
            return t

        # ======================= layers =======================
        for l in range(L):
            ln1g = load_vec(lnp, ln1g_d, l, CK, "ln1g")
            ln1b = load_vec(lnp, ln1b_d, l, CK, "ln1b")
            ab_sb = load_vec(lnp, ab_d, l, CK, "ab_sb")
            wv_sb = wvp.tile([P, CK, NHC * HS], BF16, name="wv_sb")
            nc.sync.dma_start(wv_sb[:], wv_d.ap()[l])
            phpm_sb = php.tile([P, NPR, C], BF16, name="phpm_sb")
            nc.sync.dma_start(phpm_sb[:], phpm_d.ap()[l])
            w1_sb = w1p.tile([P, CK, FS], BF16, name="w1_sb")
            nc.sync.dma_start(w1_sb[:], w1_d.ap()[l])
            with tc.tile_pool(name=f"ot_{l}", bufs=1) as otp:
                OTh = [otp.tile([P, NPR, TH], BF16, name=f"OT{i}")
                       for i in range(2)]
                with tc.tile_pool(name=f"xn_{l}", bufs=1) as xnp:
                    xnh = [xnp.tile([P, CK, TH], BF16, name=f"xn{i}")
                           for i in range(2)]
                    with (tc.tile_pool(name=f"va_{l}", bufs=1) as vap,
                          tc.tile_pool(name=f"qk_{l}", bufs=1) as qkp):
                        vg = [vap.tile([P, NHC, HS + 1], BF16, name=f"vg{g}")
                              for g in range(GPT)]
                        qTa = [[qkp.tile([P, TH], BF16, name=f"qT{pr}_{th}")
                                for th in range(2)] for pr in range(NPR)]
                        kTa = [[qkp.tile([P, TH], BF16, name=f"kT{pr}_{th}")
                                for th in range(2)] for pr in range(NPR)]
                        # th-major: all th0 work is emitted before anything
                        # depending on th1's residual (hides the th1 FFN AR)
                        with (tc.tile_pool(name=f"vp_{l}", bufs=2,
                                           space="PSUM") as vps,
                              tc.tile_pool(name=f"qp_{l}", bufs=2,
                                           space="PSUM") as qps):
                            for th in range(2):
                                ln_half(xnh[th], slice(0, TH),
                                        xTh[th], ln1g, ln1b, lnw, lnwp)
                                for g in range(4 * th, 4 * th + 4):
                                    lg = (g % 4) * P
                                    nc.vector.memset(
                                        vg[g][:, :, HS:HS + 1], 1.0)
                                    vp = vps.tile([P, NHC * HS], F32,
                                                  name="vp")
                                    for k in range(CK):
                                        nc.tensor.matmul(
                                            vp[:], xnh[th][:, k, lg:lg + P],
                                            wv_sb[:, k], start=(k == 0),
                                            stop=(k == CK - 1))
                                    nc.scalar.activation(
                                        vg[g][:, :, 0:HS],
                                        vp[:].rearrange("p (h s) -> p h s",
                                                        h=NHC),
                                        AF.Copy)
                                for pr in range(NPR):
                                    wq_p = wst.tile([P, CK, P], BF16,
                                                    name="wq_p")
                                    nc.sync.dma_start(wq_p[:],
                                                      wq_d.ap()[l, pr])
                                    wk_p = wst.tile([P, CK, P], BF16,
                                                    name="wk_p")
                                    nc.sync.dma_start(wk_p[:],
                                                      wk_d.ap()[l, pr])
                                    qp = qps.tile([P, TH], F32, name="qp")
                                    kp = qps.tile([P, TH], F32, name="kp")
                                    for k in range(CK):
                                        nc.tensor.matmul(
                                            qp[:], wq_p[:, k],
                                            xnh[th][:, k, :],
                                            start=(k == 0),
                                            stop=(k == CK - 1))
                                        nc.tensor.matmul(
                                            kp[:], wk_p[:, k],
                                            xnh[th][:, k, :],
                                            start=(k == 0),
                                            stop=(k == CK - 1))
                                    nc.scalar.activation(qTa[pr][th][:],
                                                         qp[:], AF.Copy)
                                    nc.vector.tensor_copy(kTa[pr][th][:],
                                                          kp[:])
                        if l == 0:
                            tap("t_xn1", xnh[0][:, 0, :8])
                            tap("t_v0", vg[0][:, 0, :8])
                            tap("t_q0", qTa[0][0][:, :8])
                            tap("t_k0", kTa[0][0][:, :8])

                        # ---- attention
                        with (tc.tile_pool(name=f"at_{l}", bufs=2) as atp,
                              tc.tile_pool(name=f"ap_{l}", bufs=1,
                                           space="PSUM") as aps):
                            for pr in range(NPR):
                                qTh = qTa[pr]
                                kTh = kTa[pr]
                                for h01 in range(2):
                                    off = h01 * HS
                                    h = 2 * pr + h01
                                    o_ps = [aps.tile([HS + 1, TH], F32,
                                                     name=f"o_ps{th}")
                                            for th in range(2)]
                                    for j in range(GPT):
                                        kblk = kTh[j // 4][off:off + HS,
                                                           (j % 4) * P:
                                                           (j % 4 + 1) * P]
                                        for th in range(2):
                                            t0 = th * TH
                                            if t0 + TH <= j * P:
                                                continue
                                            lo = max(t0, j * P)
                                            n = t0 + TH - lo
                                            ll = lo - t0
                                            wp = aps.tile([P, TH], F32,
                                                          name="wp", bufs=2)
                                            nc.tensor.matmul(
                                                wp[:, :n], kblk,
                                                qTh[th][off:off + HS, ll:TH],
                                                start=True, stop=True)
                                            if lo == j * P:
                                                nc.vector.tensor_add(
                                                    wp[:, :P],
                                                    wp[:, :P], m0[:])
                                            ew = atp.tile([P, TH], BF16,
                                                          name="ew")
                                            nc.scalar.activation(
                                                ew[:, ll:TH],
                                                wp[:, :n], AF.Exp,
                                                scale=SCALE)
                                            nc.tensor.matmul(
                                                o_ps[th][:, ll:TH],
                                                vg[j][:, h, :],
                                                ew[:, ll:TH],
                                                start=(j == 0),
                                                stop=(j == (3 if th == 0
                                                            else 7)))
                                            if (l == 0 and h == 0 and j == 0
                                                    and th == 0):
                                                tap("t_ew", ew[:, :8])
                                    for th in range(2):
                                        a = atp.tile([HS + 1, TH], F32,
                                                     name="a")
                                        nc.scalar.activation(a[:], o_ps[th][:],
                                                             AF.Copy)
                                        if (l == 0 and pr == 0 and h01 == 0
                                                and th == 0):
                                            tap("t_den", a[HS:HS + 1, :16])
                                        rd0 = atp.tile([1, TH], F32,
                                                       name="rd0")
                                        nc.sync.dma_start(rd0[:],
                                                          a[HS:HS + 1])
                                        rdr = atp.tile([1, TH], F32,
                                                       name="rdr")
                                        nc.vector.reciprocal_approx_fast(
                                            rdr[:], rd0[:])
                                        rbs = atp.tile([HS, TH], F32,
                                                       name="rbs")
                                        nc.gpsimd.partition_broadcast(
                                            rbs[:], rdr[:])
                                        if (l == 0 and pr == 0 and h01 == 0
                                                and th == 0):
                                            tap("t_rcp", rbs[0:1, :16])
                                        with nc.allow_low_precision(
                                                reason="bf16 attn out"):
                                            if h01 == 0:
                                                nc.vector.tensor_mul(
                                                    OTh[th][0:HS, pr, :],
                                                    a[0:HS], rbs[:])
                                            else:
                                                otmp = atp.tile([HS, TH], BF16,
                                                                name="otmp")
                                                nc.vector.tensor_mul(
                                                    otmp[:], a[0:HS], rbs[:])
                                                nc.sync.dma_start(
                                                    OTh[th][HS:P, pr, :],
                                                    otmp[:])
                        if l == 0:
                            tap("t_ot", OTh[0][:, 0, :8])

                # ---- sa = OT.T @ phpm (+bias), CC, residual
                with tc.tile_pool(name=f"sp_{l}", bufs=2,
                                  space="PSUM") as sps:
                    for th in range(2):
                        bin_ = drp.tile([P, CK, TH], F16, name="cc_in")
                        bout = drp.tile([P, CK, TH], F16, name="cc_out")
                        for ct in range(CK):
                            sp = sps.tile([P, TH], F32, name="sp")
                            for o in range(NPR):
                                nc.tensor.matmul(
                                    sp[:], phpm_sb[:, o, ct * P:(ct + 1) * P],
                                    OTh[th][:, o, :],
                                    start=(o == 0), stop=(o == NPR - 1))
                            sa_c = sap.tile([P, TH], F16, name="sa_c")
                            with nc.allow_low_precision(reason="f16 cc"):
                                nc.vector.tensor_scalar(
                                    sa_c[:], sp[:], ab_sb[:, ct:ct + 1],
                                    None, ALU.add)
                            nc.sync.dma_start(bin_[:, ct], sa_c[:])
                        nc.gpsimd.collective_compute(
                            "AllReduce", ALU.add, replica_groups=GROUPS,
                            ins=[bin_.opt()], outs=[bout.opt()])
                        for ct in range(CK):
                            ar_c = sap.tile([P, TH], F16, name="ar_c")
                            nc.sync.dma_start(ar_c[:], bout[:, ct])
                            if l == 0 and th == 0 and ct == 0:
                                tap("t_sa", ar_c[:, :8])
                            arf = sap.tile([P, TH], F32, name="arf")
                            nc.vector.tensor_copy(arf[:], ar_c[:])
                            nc.vector.tensor_add(xTh[th][:, ct, :],
                                                 xTh[th][:, ct, :], arf[:])
            if l == 0:
                tap("t_x1", xTh[0][:, 0, :16].bitcast(F32))

            # ---- LN2 + FFN (th-split)
            ln2g = load_vec(lnp, ln2g_d, l, CK, "ln2g")
            ln2b = load_vec(lnp, ln2b_d, l, CK, "ln2b")
            b2g = load_vec(lnp, b2g_d, l, CK, "b2g")
            b1_sb = load_vec(lnp, b1s_d, l, FCK, "b1_sb")
            with tc.tile_pool(name=f"f_{l}", bufs=2) as fsb:
                for th in range(2):
                    xn2 = fsb.tile([P, CK, TH], BF16, name="xn2")
                    ln_half(xn2, slice(0, TH), xTh[th], ln2g, ln2b,
                            lnw, lnwp)
                    if l == 0 and th == 0:
                        tap("t_xn2", xn2[:, 0, :8])
                    hT = fsb.tile([P, FCK, TH], BF16, name="hT")
                    with tc.tile_pool(name=f"hp_{l}_{th}", bufs=1,
                                      space="PSUM") as hps:
                        for fq in range(4):
                            h_ps = [hps.tile([P, TH], F32, name=f"h{i}")
                                    for i in range(4)]
                            for k in range(CK):
                                for ft in range(4):
                                    fcol = fq * TH + ft * P
                                    nc.tensor.matmul(
                                        h_ps[ft][:],
                                        w1_sb[:, k, fcol:fcol + P],
                                        xn2[:, k], start=(k == 0),
                                        stop=(k == CK - 1))
                            for ft in range(4):
                                fc = fq * 4 + ft
                                nc.scalar.activation(
                                    hT[:, fc], h_ps[ft][:], AF.Gelu,
                                    bias=b1_sb[:, fc:fc + 1])
                    if l == 0 and th == 0:
                        tap("t_h", hT[:, 0, :8])
                    bin2 = drp.tile([P, CK, TH], F16, name="cc_in")
                    bout2 = drp.tile([P, CK, TH], F16, name="cc_out")
                    with tc.tile_pool(name=f"wp_{l}_{th}", bufs=2,
                                      space="PSUM") as wps:
                        for ct in range(CK):
                            w2_ct = w2p.tile([P, FCK, P], BF16, name="w2_ct")
                            nc.sync.dma_start(w2_ct[:], w2_d.ap()[l, ct])
                            fp = wps.tile([P, TH], F32, name="fp")
                            for fc in range(FCK):
                                nc.tensor.matmul(
                                    fp[:], w2_ct[:, fc], hT[:, fc],
                                    start=(fc == 0),
                                    stop=(fc == FCK - 1))
                            fo_c = fcp.tile([P, TH], F16, name="sa_c")
                            with nc.allow_low_precision(reason="f16 cc"):
                                nc.vector.tensor_scalar(
                                    fo_c[:], fp[:], b2g[:, ct:ct + 1], None,
                                    ALU.add)
                            nc.sync.dma_start(bin2[:, ct], fo_c[:])
                    nc.gpsimd.collective_compute(
                        "AllReduce", ALU.add, replica_groups=GROUPS,
                        ins=[bin2.opt()], outs=[bout2.opt()])
                    for ct in range(CK):
                        ar_c = fcp.tile([P, TH], F16, name="ar_c")
                        nc.sync.dma_start(ar_c[:], bout2[:, ct])
                        if l == 0 and th == 0 and ct == 0:
                            tap("t_ffn", ar_c[:, :8])
                        arf = fcp.tile([P, TH], F32, name="arf")
                        nc.vector.tensor_copy(arf[:], ar_c[:])
                        nc.vector.tensor_add(xTh[th][:, ct, :],
                                             xTh[th][:, ct, :], arf[:])
            if l == 0:
                tap("t_x2", xTh[0][:, 0, :16].bitcast(F32))
            tap(f"t_xl{l}", xTh[0][:, 0, :16].bitcast(F32))

        # ======================= final LN + head =======================
        with tc.tile_pool(name="xf", bufs=1) as xfp:
            xfT = xfp.tile([P, CK, T], BF16)
            with tc.tile_pool(name="fin", bufs=1) as fin:
                lnfg = load_vec(fin, lnfg_d, None, CK, "lnfg")
                lnfb = load_vec(fin, lnfb_d, None, CK, "lnfb")
                for th in range(2):
                    ln_half(xfT, slice(th * TH, (th + 1) * TH), xTh[th],
                            lnfg, lnfb, lnw, lnwp)
            tap("t_xf", xfT[:, 0, :8])
            with (tc.tile_pool(name="hw", bufs=3) as hwp,
                  tc.tile_pool(name="lg", bufs=3) as lgp,
                  tc.tile_pool(name="hbp", bufs=1) as hbp,
                  tc.tile_pool(name="lp2", bufs=3, space="PSUM") as lps2,
                  tc.tile_pool(name="bp2", bufs=2, space="PSUM") as bps):
                for vt in range(NVT):
                    vs = slice(vt * VT, (vt + 1) * VT)
                    hb_sb = hbp.tile([1, VT], F32R, name="hb_sb",
                                     bufs=2)
                    nc.sync.dma_start(hb_sb[:],
                                      hb_d.ap()[:, vs].bitcast(F32R))
                    hw_c = hwp.tile([P, CK, VT], BF16, name="hw_c")
                    nc.sync.dma_start(
                        hw_c[:],
                        hw_d.ap()[:, :, vs].rearrange("k p v -> p k v"))
                    bp = bps.tile([P, VT], F32, name="bp")
                    nc.tensor.matmul(bp[:], ones_r[:1, :], hb_sb[:],
                                     start=True, stop=True)
                    bs = lgp.tile([P, VT], F32, name="bs")
                    nc.scalar.activation(bs[:], bp[:], AF.Copy)
                    for tt in range(T // P):
                        lp = lps2.tile([P, VT], F32, name="lp")
                        for k in range(CK):
                            nc.tensor.matmul(
                                lp[:], xfT[:, k, tt * P:(tt + 1) * P],
                                hw_c[:, k], start=(k == 0), stop=(k == CK - 1))
                        lg_sb = lgp.tile([P, VT], BF16, name="lg_sb")
                        with nc.allow_low_precision(reason="bf16 logits"):
                            nc.vector.tensor_add(lg_sb[:], lp[:], bs[:])
                        if vt == 0 and tt == 0:
                            tap("t_lg", lg_sb[:, :8])
                        nc.sync.dma_start(
                            logits_d.ap()[tt * P:(tt + 1) * P, vs], lg_sb[:])

    nc.compile()
    return nc


# ----------------------------------------------------------------------------
# host entry
# ----------------------------------------------------------------------------

def kernel(**inputs):
    from concourse.bass_utils import run_bass_kernel_spmd

    if "nc" not in _CACHE:
        _CACHE["nc"] = build_nc(taps=False)
    nc = _CACHE["nc"]

    # weights depend only on the TP half; share arrays across DP groups
    wd = [_prep_core(inputs, tp) for tp in range(2)]
    in_maps = []
    for c in range(8):
        b, tp = c // 2, c % 2
        m = dict(wd[tp])
        m["idx"] = np.ascontiguousarray(
            np.asarray(inputs["idx"][b]).astype(np.int32).reshape(T, 1))
        in_maps.append(m)
    res = run_bass_kernel_spmd(nc, in_maps, core_ids=list(range(8)))
    out = np.zeros((B, T, V), np.float32)
    for c in range(8):
        b, tp = c // 2, c % 2
        out[b, :, tp * VS:(tp + 1) * VS] = np.asarray(
            res.results[c]["logits"], dtype=np.float32)
    return out



# revision 22
# speedup vs baseline: 1.1814x; 1.1814x over previous
"""Trainium2 Bass kernel for an 8-layer dense transformer (B=4,T=1024,C=1024,
H=16,HS=64,V=32000), sharded over 8 NeuronCores as DP=4 (batch) x TP=2
(heads/FFN-hidden/vocab), with pairwise AllReduce after the attention
projection and FFN down-projection.

v3: warm-PE restructure. Two token-half streams pipelined through
attention/proj/FFN so scalar-heavy softmax overlaps PE-heavy GEMMs; LN
gains/betas folded into weights on host (device LN is pure (x-m)*rstd);
LN sums via column-tiled concurrent PE matmuls, broadcasts via K=1 PE
matmuls into PSUM; causal mask applied by zeroing exp-weights on GPSIMD
(SBUF) instead of -inf adds on Vector (PSUM); head-pair score matmuls
row-tiled to run concurrently; scores emitted interleaved with QK so
exp starts early; per-layer weight DMAs issued up front.
"""
import numpy as np
import ml_dtypes

import concourse.bass as bass
import concourse.mybir as mybir
import concourse.tile as tile
from concourse import bacc
from concourse.masks import make_identity

V, B, T, C, H, L = 32000, 4, 1024, 1024, 16, 8
HS = C // H          # 64
P = 128
NHC = H // 2         # heads per core = 8
NPR = NHC // 2       # head pairs = 4
CK = C // P          # channel chunks = 8
F = 4 * C
FS = F // 2          # ffn shard = 2048
FCK = FS // P        # f chunks = 16
VS = V // 2          # vocab shard = 16000
VT = 500             # vocab tile
NVT = VS // VT       # 32
TH = 512             # token half
GPT = T // P         # token groups = 8
SCALE = float(C) ** -0.5
EPS = 1e-5

F32 = mybir.dt.float32
F32R = mybir.dt.float32r
BF16 = mybir.dt.bfloat16
F16 = mybir.dt.float16
I32 = mybir.dt.int32
AF = mybir.ActivationFunctionType
ALU = mybir.AluOpType
GROUPS = [[0, 1], [2, 3], [4, 5], [6, 7]]
BF = ml_dtypes.bfloat16

_CACHE = {}


# ----------------------------------------------------------------------------
# host-side shard prep
# ----------------------------------------------------------------------------

def _prep_core(inp, core):
    b, tp = core // 2, core % 2
    heads = slice(tp * NHC, (tp + 1) * NHC)
    f32 = lambda a: np.ascontiguousarray(np.asarray(a, np.float32))
    bf = lambda a: np.ascontiguousarray(np.asarray(a).astype(BF))
    d = {}
    d["idx"] = np.ascontiguousarray(np.asarray(inp["idx"][b])
                                    .astype(np.int32).reshape(T, 1))
    d["tok_emb"] = bf(inp["tok_emb"])
    d["pos_emb"] = bf(inp["pos_emb"])

    ln1g = np.asarray(inp["ln1_g"], np.float32)   # [L, C]
    ln1b = np.asarray(inp["ln1_b"], np.float32)
    ln2g = np.asarray(inp["ln2_g"], np.float32)
    ln2b = np.asarray(inp["ln2_b"], np.float32)
    lnfg = np.asarray(inp["lnf_g"], np.float32)   # [C]
    lnfb = np.asarray(inp["lnf_b"], np.float32)

    # Wq/Wk with ln1_g folded; per-pair layout [L, P(c in chunk), NPR, CK, P(s)]
    qb = np.zeros((L, P, NPR), np.float32)
    kb = np.zeros((L, P, NPR), np.float32)
    for nm, bias in (("Wq", qb), ("Wk", kb)):
        w = np.asarray(inp[nm], np.float32)[:, heads]          # [L,8,C,HS]
        bias[:] = np.einsum("lhcs,lc->lhs", w, ln1b).reshape(
            L, NPR, 2 * HS).transpose(0, 2, 1)
        w = w * ln1g[:, None, :, None]
        w = w.reshape(L, NPR, 2, C, HS).transpose(0, 1, 3, 2, 4)
        w = w.reshape(L, NPR, CK, P, 2 * HS)                   # [L,NPR,CK,P,128]
        d[nm.lower()] = bf(np.ascontiguousarray(
            w.transpose(0, 3, 1, 2, 4)).reshape(L, P, NPR * CK * P))
    d["qb"] = np.ascontiguousarray(qb)
    d["kb"] = np.ascontiguousarray(kb)

    wv_raw = np.asarray(inp["Wv"], np.float32)[:, heads]       # [L,8,C,HS]
    vb = np.einsum("lhcs,lc->lhs", wv_raw, ln1b)               # [L,8,HS]
    wv = wv_raw * ln1g[:, None, :, None]
    wv = np.transpose(wv, (0, 2, 1, 3)).reshape(L, CK, P, NHC * HS)
    d["wv"] = bf(np.transpose(wv, (0, 2, 1, 3)).reshape(L, P, CK * NHC * HS))

    # fused per-head proj + mix: phpm[l] = PhW_flat[l] @ Pm[l]  [512, C]
    phw = np.asarray(inp["Ph_w"], np.float32)[:, heads].reshape(L, NHC * HS, C)
    pm = np.asarray(inp["Pm_w"], np.float32)
    phpm = np.einsum("loc,lcd->lod", phw, pm)                  # [L, 512, C]
    phpm_r = phpm.reshape(L, NPR, P, C).transpose(0, 2, 1, 3)  # [L,P,NPR,C]
    d["phpm"] = bf(phpm_r.reshape(L, P, NPR * C))

    w1 = np.asarray(inp["W1"], np.float32)[:, :, tp * FS:(tp + 1) * FS]
    b1x = (np.asarray(inp["b1"], np.float32)[:, tp * FS:(tp + 1) * FS]
           + np.einsum("lcf,lc->lf", w1, ln2b))
    w1 = w1 * ln2g[:, :, None]
    w1 = w1.reshape(L, CK, P, 8, 2 * P).transpose(0, 3, 2, 1, 4)
    d["w1"] = bf(np.ascontiguousarray(w1).reshape(L, 8, P, CK * 2 * P))
    w2 = np.asarray(inp["W2"], np.float32)[:, tp * FS:(tp + 1) * FS]
    w2 = w2.reshape(L, FCK, P, CK, P).transpose(0, 3, 2, 1, 4)
    d["w2"] = bf(w2.reshape(L, CK, P, FCK * P))
    d["b1s"] = np.ascontiguousarray(b1x.reshape(L, FCK, P, 1))

    # attn bias: head-sum of Ph_b through Pm (+Pm_b on tp0) + V-beta fold
    phb = np.asarray(inp["Ph_b"], np.float32)[:, heads].sum(1)   # [L, C]
    ab = np.einsum("lc,lcd->ld", phb, pm)
    ab = ab + np.einsum("lhs,lhsc->lc", vb,
                        phpm.reshape(L, NHC, HS, C))
    if tp == 0:
        ab = ab + np.asarray(inp["Pm_b"], np.float32)
    d["attn_bias"] = np.ascontiguousarray(ab.reshape(L, CK, P, 1))
    b2 = (np.asarray(inp["b2"], np.float32) if tp == 0
          else np.zeros((L, C), np.float32))
    d["b2g"] = np.ascontiguousarray(b2.reshape(L, CK, P, 1))

    hw = np.asarray(inp["head_w"], np.float32)
    hb = (np.asarray(inp["head_b"], np.float32)[tp * VS:(tp + 1) * VS]
          + np.einsum("cv,c->v", hw[:, tp * VS:(tp + 1) * VS], lnfb))
    hw = hw * lnfg[:, None]
    d["headw"] = bf(hw[:, tp * VS:(tp + 1) * VS].reshape(CK, P, VS))
    d["headb"] = np.ascontiguousarray(hb.reshape(1, VS))
    return d


# ----------------------------------------------------------------------------
# device program
# ----------------------------------------------------------------------------

def build_nc(taps=False):
    nc = bacc.Bacc("TRN2", target_bir_lowering=False, debug=False, num_devices=8)

    def din(name, shape, dt_):
        return nc.dram_tensor(name, list(shape), dt_, kind="ExternalInput")

    idx_d = din("idx", [T, 1], I32)
    tok_d = din("tok_emb", [V, C], BF16)
    pos_d = din("pos_emb", [T, C], BF16)
    wq_d = din("wq", [L, P, NPR * CK * P], BF16)
    wk_d = din("wk", [L, P, NPR * CK * P], BF16)
    qb_d = din("qb", [L, P, NPR], F32)
    kb_d = din("kb", [L, P, NPR], F32)
    wv_d = din("wv", [L, P, CK * NHC * HS], BF16)
    phpm_d = din("phpm", [L, P, NPR * C], BF16)
    w1_d = din("w1", [L, 8, P, CK * 2 * P], BF16)
    w2_d = din("w2", [L, CK, P, FCK * P], BF16)
    b1s_d = din("b1s", [L, FCK, P, 1], F32)
    ab_d = din("attn_bias", [L, CK, P, 1], F32)
    b2g_d = din("b2g", [L, CK, P, 1], F32)
    hw_d = din("headw", [CK, P, VS], BF16)
    hb_d = din("headb", [1, VS], F32)

    logits_d = nc.dram_tensor("logits", [T, VS], BF16, kind="ExternalOutput")

    tap_d = {}
    if taps:
        for nm in (["t_embed", "t_xn1", "t_q0", "t_k0", "t_v0", "t_ew", "t_ot",
                    "t_sa", "t_x1", "t_xn2", "t_h", "t_ffn", "t_x2",
                    "t_xf", "t_lg", "t_den", "t_rcp"]
                   + [f"t_xl{i}" for i in range(L)]):
            tap_d[nm] = nc.dram_tensor(nm, [P, 16], F32, kind="ExternalOutput")

    from contextlib import ExitStack
    with tile.TileContext(nc) as tc, ExitStack() as st:
        # ------- static pools
        cst = st.enter_context(tc.tile_pool(name="cst", bufs=1))
        xtp = st.enter_context(tc.tile_pool(name="xtp", bufs=1))
        wqk = st.enter_context(tc.tile_pool(name="wqk", bufs=1))
        wvp = st.enter_context(tc.tile_pool(name="wvp", bufs=1))
        php = st.enter_context(tc.tile_pool(name="php", bufs=1))
        w1p = st.enter_context(tc.tile_pool(name="w1p", bufs=1))
        w2p = st.enter_context(tc.tile_pool(name="w2p", bufs=2))
        lnp = st.enter_context(tc.tile_pool(name="lnp", bufs=2))
        sqp = st.enter_context(tc.tile_pool(name="sqp", bufs=1))
        tpp = st.enter_context(tc.tile_pool(name="tpp", bufs=2))
        lns = st.enter_context(tc.tile_pool(name="lns", bufs=2))
        sap = st.enter_context(tc.tile_pool(name="sap", bufs=2))
        drp = st.enter_context(tc.tile_pool(name="drp", bufs=4, space="DRAM"))

        # residual stream, split by token half: [p, k, t-local]
        xTh = [xtp.tile([P, CK, TH], F32R, name=f"xT{i}") for i in range(2)]
        xnh = [xtp.tile([P, CK, TH], BF16, name=f"xn{i}") for i in range(2)]

        ones_f = cst.tile([P, P], F32)
        nc.vector.memset(ones_f[:], 1.0)
        ones_r = cst.tile([P, P], F32R)
        nc.vector.tensor_copy(ones_r[:], ones_f[:])
        oC_f = cst.tile([P, 1], F32)
        nc.vector.memset(oC_f[:], 1.0 / C)
        oC_r = cst.tile([P, 1], F32R)
        nc.vector.tensor_copy(oC_r[:], oC_f[:])
        oC_b = cst.tile([P, 1], BF16)
        nc.vector.tensor_copy(oC_b[:], oC_f[:])
        ident = cst.tile([P, P], F32)
        make_identity(nc, ident[:])
        ident_b = cst.tile([P, P], BF16)
        nc.vector.tensor_copy(ident_b[:], ident[:])
        eps_b = cst.tile([P, 1], F32)
        nc.vector.memset(eps_b[:], EPS)

        def tap(nm, src_ap):
            if not taps:
                return
            n = src_ap.free_size()
            p = src_ap.shape[0]
            if src_ap.dtype != F32:
                tmp = tpp.tile([P, 16], F32, name="tapt")
                nc.vector.tensor_copy(tmp[:p, :n], src_ap)
                src_ap = tmp[:p, :n]
            nc.sync.dma_start(tap_d[nm].ap()[:p, :n], src_ap)

        # ------- embedding: gather + pos, transpose into xTh
        with (tc.tile_pool(name="emb", bufs=3) as emb,
              tc.tile_pool(name="embp", bufs=4, space="PSUM") as embp):
            idx_sb = emb.tile([P, GPT], I32, name="idx_sb")
            nc.sync.dma_start(idx_sb[:],
                              idx_d.ap().rearrange("(g p) o -> p (g o)", p=P))
            for g in range(GPT):
                th, lg = g // 4, (g % 4) * P
                ge = emb.tile([P, C], BF16, name="ge")
                nc.gpsimd.indirect_dma_start(
                    out=ge[:], out_offset=None, in_=tok_d.ap(),
                    in_offset=bass.IndirectOffsetOnAxis(ap=idx_sb[:, g:g + 1],
                                                        axis=0))
                pe = emb.tile([P, C], BF16, name="pe")
                nc.sync.dma_start(pe[:], pos_d.ap()[g * P:(g + 1) * P, :])
                nc.vector.tensor_add(ge[:], ge[:], pe[:])
                for k in range(CK):
                    pt = embp.tile([P, P], BF16, name="pt")
                    nc.tensor.transpose(pt[:], ge[:, k * P:(k + 1) * P],
                                        ident_b[:])
                    nc.scalar.activation(xTh[th][:, k, lg:lg + P], pt[:],
                                         AF.Copy)
        tap("t_embed", xTh[0][:, 0, :16].bitcast(F32))

        # PSUM budget (8 banks): sp(1) + bc(2) + mA(2) + mB(2) + o_ps(1)
        psp = st.enter_context(tc.tile_pool(name="psp", bufs=1, space="PSUM"))
        bcp = st.enter_context(tc.tile_pool(name="bcp", bufs=2, space="PSUM"))
        mmp = st.enter_context(tc.tile_pool(name="mmp", bufs=2, space="PSUM"))
        opp = st.enter_context(tc.tile_pool(name="opp", bufs=1, space="PSUM"))

        # ------- layernorm of one token-half: dst = (src - mean) * rstd (bf16)
        # stats via concurrent col-tiled PE sums; broadcasts via K=1 matmuls.
        def ln_half(dst, dst_ts, src):
            sp = psp.tile([P, TH], F32, name="sp")
            with nc.allow_low_precision(reason="bf16 x^2 for var"):
                for k in range(CK):
                    sqb = sqp.tile([P, TH], BF16, name="sqb", bufs=3)
                    nc.gpsimd.tensor_mul(sqb[:], src[:, k], src[:, k])
                    nc.tensor.matmul(sp[0:1, :], oC_r[:, :1], src[:, k],
                                     start=(k == 0), stop=(k == CK - 1))
                    nc.tensor.matmul(sp[32:33, :], oC_b[:, :1], sqb[:],
                                     start=(k == 0), stop=(k == CK - 1))
            mean_sb = lns.tile([1, TH], F32, name="lt", bufs=4)
            nc.scalar.activation(mean_sb[:], sp[0:1, :], AF.Copy)
            m2 = lns.tile([1, TH], F32, name="lt", bufs=4)
            nc.vector.tensor_mul(m2[:], mean_sb[:], mean_sb[:])
            var = lns.tile([1, TH], F32, name="lt", bufs=4)
            nc.vector.tensor_sub(var[:], sp[32:33, :], m2[:])
            std = lns.tile([1, TH], F32, name="lt", bufs=4)
            nc.scalar.activation(std[:], var[:], AF.Sqrt, bias=eps_b[0:1, :1])
            rstd = lns.tile([1, TH], F32, name="rstd", bufs=2)
            nc.vector.reciprocal_approx_fast(rstd[:], std[:])
            bm = bcp.tile([P, TH], F32, name="bc")
            nc.tensor.matmul(bm[:], ones_f[0:1, :], mean_sb[:],
                             start=True, stop=True)
            br = bcp.tile([P, TH], F32, name="bc")
            nc.tensor.matmul(br[:], ones_f[0:1, :], rstd[:],
                             start=True, stop=True)
            for k in range(CK):
                t1 = lns.tile([P, TH], F32R, name="t1", bufs=2)
                nc.vector.tensor_sub(t1[:], src[:, k], bm[:])
                with nc.allow_low_precision(reason="bf16 ln out"):
                    nc.vector.tensor_mul(dst[:, k, dst_ts], t1[:], br[:])

        def load_vec(pool, dram, l, n, name):
            t = pool.tile([P, n], F32, name=name)
            src = dram.ap()[l] if l is not None else dram.ap()
            nc.sync.dma_start(t[:], src.rearrange("k p o -> p (k o)"))
            return t

        # ======================= layers =======================
        for l in range(L):
            # ---- layer-resident weights (issued up front, big DMAs)
            wq_sb = wqk.tile([P, NPR, CK, P], BF16, name="wq_sb")
            nc.sync.dma_start(wq_sb[:], wq_d.ap()[l])
            wk_sb = wqk.tile([P, NPR, CK, P], BF16, name="wk_sb")
            nc.sync.dma_start(wk_sb[:], wk_d.ap()[l])
            wv_sb = wvp.tile([P, CK, NHC * HS], BF16, name="wv_sb")
            nc.sync.dma_start(wv_sb[:], wv_d.ap()[l])
            phpm_sb = php.tile([P, NPR, C], BF16, name="phpm_sb")
            nc.sync.dma_start(phpm_sb[:], phpm_d.ap()[l])
            qb_sb = lnp.tile([P, NPR], F32, name="qb_sb")
            nc.sync.dma_start(qb_sb[:], qb_d.ap()[l])
            kb_sb = lnp.tile([P, NPR], F32, name="kb_sb")
            nc.sync.dma_start(kb_sb[:], kb_d.ap()[l])
            ab_sb = load_vec(lnp, ab_d, l, CK, "ab_sb")
            b1_sb = load_vec(lnp, b1s_d, l, FCK, "b1_sb")
            b2g = load_vec(lnp, b2g_d, l, CK, "b2g")

            with (tc.tile_pool(name=f"qk_{l}", bufs=1) as qkp,
                  tc.tile_pool(name=f"va_{l}", bufs=1) as vap,
                  tc.tile_pool(name=f"ot_{l}", bufs=1) as otp,
                  tc.tile_pool(name=f"at_{l}", bufs=2) as atp,
                  tc.tile_pool(name=f"ew_{l}", bufs=16) as ewp,
                  tc.tile_pool(name=f"f_{l}", bufs=1) as fsb):
                qTa = [[None] * 2 for _ in range(NPR)]
                kTa = [[None] * 2 for _ in range(NPR)]
                vg = [None] * GPT
                OTh = [otp.tile([P, NPR, TH], BF16, name=f"OT{i}")
                       for i in range(2)]

                def qkv_attn(th):
                    """LN1, V, then per head-pair: QK -> scores -> AV -> OT.

                    Scores for the two heads of a pair are emitted adjacent
                    so the K=64 row-tiled matmuls run concurrently; exp lags
                    one j-block behind on the scalar engine; AV consumes the
                    ew ring as it fills.
                    """
                    ln_half(xnh[th], slice(0, TH), xTh[th])
                    xn = xnh[th]
                    t0 = th * TH
                    jmax = 4 * th + 4
                    for g in range(4 * th, 4 * th + 4):
                        lg = (g % 4) * P
                        vp = mmp.tile([P, NHC * HS], F32, name="mA")
                        for k in range(CK):
                            nc.tensor.matmul(vp[:], xn[:, k, lg:lg + P],
                                             wv_sb[:, k], start=(k == 0),
                                             stop=(k == CK - 1))
                        vt = vap.tile([P, NHC, HS + 1], BF16, name=f"vg{g}")
                        nc.vector.memset(vt[:, :, HS:HS + 1], 1.0)
                        nc.scalar.activation(
                            vt[:, :, 0:HS],
                            vp[:].rearrange("p (h s) -> p h s", h=NHC),
                            AF.Copy)
                        vg[g] = vt
                    for pr in range(NPR):
                        qp = mmp.tile([P, TH], F32, name="mA")
                        kp = mmp.tile([P, TH], F32, name="mB")
                        for k in range(CK):
                            nc.tensor.matmul(qp[:], wq_sb[:, pr, k], xn[:, k],
                                             start=(k == 0), stop=(k == CK - 1))
                            nc.tensor.matmul(kp[:], wk_sb[:, pr, k], xn[:, k],
                                             start=(k == 0), stop=(k == CK - 1))
                        qT = qkp.tile([P, TH], BF16, name="qT", bufs=2)
                        with nc.allow_low_precision(reason="bf16 q"):
                            nc.vector.tensor_scalar(
                                qT[:], qp[:], qb_sb[:, pr:pr + 1], None,
                                ALU.add)
                        kT = qkp.tile([P, TH], BF16, name=f"kT{pr}_{th}")
                        with nc.allow_low_precision(reason="bf16 k"):
                            nc.vector.tensor_scalar(
                                kT[:], kp[:], kb_sb[:, pr:pr + 1], None,
                                ALU.add)
                        qTa[pr][th] = qT
                        kTa[pr][th] = kT
                        # scores, both heads adjacent for row-tile concurrency
                        ews = {}
                        for j in range(jmax):
                            lo = max(t0, j * P)
                            n = t0 + TH - lo
                            ll = lo - t0
                            for h01 in range(2):
                                off = h01 * HS
                                kblk = kTa[pr][j // 4][
                                    off:off + HS, (j % 4) * P:(j % 4 + 1) * P]
                                wp = mmp.tile([P, TH], F32,
                                              name=("mA" if h01 == 0
                                                    else "mB"))
                                nc.tensor.matmul(
                                    wp[:, :n], kblk,
                                    qTa[pr][th][off:off + HS, ll:TH],
                                    start=True, stop=True)
                                ew = ewp.tile([P, TH], BF16, name="ew")
                                nc.scalar.activation(ew[:, :n], wp[:, :n],
                                                     AF.Exp, scale=SCALE)
                                if lo == j * P:
                                    nc.gpsimd.affine_select(
                                        out=ew[:, :P], in_=ew[:, :P],
                                        compare_op=ALU.is_ge, fill=0.0,
                                        base=0, pattern=[[1, P]],
                                        channel_multiplier=-1)
                                ews[(j, h01)] = (ew, ll, n)
                                if l == 0 and pr == 0 and th == 0 and j == 0 \
                                        and h01 == 0:
                                    tap("t_ew", ew[:, :8])
                        # AV + denominator + normalize per head
                        for h01 in range(2):
                            h = 2 * pr + h01
                            o_ps = opp.tile([HS + 1, TH], F32, name="o_ps")
                            for j in range(jmax):
                                ew, ll, n = ews[(j, h01)]
                                nc.tensor.matmul(
                                    o_ps[:, ll:TH], vg[j][:, h, :],
                                    ew[:, :n],
                                    start=(j == 0), stop=(j == jmax - 1))
                            a = atp.tile([HS + 1, TH], F32, name="a")
                            nc.vector.tensor_copy(a[:], o_ps[:])
                            rd0 = atp.tile([1, TH], F32, name="rd0")
                            nc.sync.dma_start(rd0[:], a[HS:HS + 1, :])
                            rdr = atp.tile([1, TH], F32, name="rdr")
                            nc.vector.reciprocal_approx_fast(rdr[:], rd0[:])
                            rbs = bcp.tile([HS, TH], F32, name="bc")
                            nc.tensor.matmul(rbs[:], ones_f[0:1, 0:HS],
                                             rdr[:], start=True, stop=True)
                            with nc.allow_low_precision(reason="bf16 attn"):
                                if h01 == 0:
                                    nc.vector.tensor_mul(
                                        OTh[th][0:HS, pr, :], a[0:HS], rbs[:])
                                else:
                                    otmp = atp.tile([HS, TH], BF16,
                                                    name="otmp")
                                    nc.vector.tensor_mul(otmp[:], a[0:HS],
                                                         rbs[:])
                                    nc.sync.dma_start(OTh[th][HS:P, pr, :],
                                                      otmp[:])
                            if l == 0 and h == 0 and th == 0:
                                tap("t_den", a[HS:HS + 1, :16])
                                tap("t_rcp", rdr[0:1, :16])
                    if l == 0 and th == 0:
                        tap("t_ot", OTh[0][:, 0, :8])

                def proj_ar(th):
                    """sa = OT.T @ phpm (+bias) -> f16 -> AllReduce."""
                    bin_ = drp.tile([P, CK, TH], F16, name="cc_in")
                    bout = drp.tile([P, CK, TH], F16, name="cc_out")
                    for ct in range(CK):
                        sp2 = mmp.tile([P, TH], F32,
                                       name=("mA" if ct % 2 == 0 else "mB"))
                        for o in range(NPR):
                            nc.tensor.matmul(
                                sp2[:], phpm_sb[:, o, ct * P:(ct + 1) * P],
                                OTh[th][:, o, :],
                                start=(o == 0), stop=(o == NPR - 1))
                        sa_c = sap.tile([P, TH], F16, name="sa_c")
                        with nc.allow_low_precision(reason="f16 cc"):
                            nc.vector.tensor_scalar(
                                sa_c[:], sp2[:], ab_sb[:, ct:ct + 1],
                                None, ALU.add)
                        nc.sync.dma_start(bin_[:, ct], sa_c[:])
                    nc.gpsimd.collective_compute(
                        "AllReduce", ALU.add, replica_groups=GROUPS,
                        ins=[bin_.opt()], outs=[bout.opt()])
                    return bout

                def resid(th, bout, tapnm=None):
                    for ct in range(CK):
                        ar_c = sap.tile([P, TH], F16, name="ar_c")
                        nc.sync.dma_start(ar_c[:], bout[:, ct])
                        if tapnm and ct == 0:
                            tap(tapnm, ar_c[:, :8])
                        arf = sap.tile([P, TH], F32, name="arf")
                        nc.vector.tensor_copy(arf[:], ar_c[:])
                        nc.vector.tensor_add(xTh[th][:, ct, :],
                                             xTh[th][:, ct, :], arf[:])

                def ffn_ar(th):
                    xn2 = xnh[th]   # reuse: QKV of this half is fully done
                    ln_half(xn2, slice(0, TH), xTh[th])
                    if l == 0 and th == 0:
                        tap("t_xn2", xn2[:, 0, :8])
                    hT = fsb.tile([P, FCK, TH], BF16, name="hT")
                    for fq in range(8):
                        w1_t = w1p.tile([P, CK, 2 * P], BF16, name="w1_t",
                                        bufs=3)
                        nc.sync.dma_start(w1_t[:], w1_d.ap()[l, fq])
                        h_ps = [mmp.tile([P, TH], F32, name=nm)
                                for nm in ("mA", "mB")]
                        for k in range(CK):
                            for ft in range(2):
                                nc.tensor.matmul(
                                    h_ps[ft][:], w1_t[:, k, ft * P:(ft + 1) * P],
                                    xn2[:, k], start=(k == 0),
                                    stop=(k == CK - 1))
                        for ft in range(2):
                            fc = fq * 2 + ft
                            nc.scalar.activation(
                                hT[:, fc], h_ps[ft][:], AF.Gelu,
                                bias=b1_sb[:, fc:fc + 1])
                    if l == 0 and th == 0:
                        tap("t_h", hT[:, 0, :8])
                    bin2 = drp.tile([P, CK, TH], F16, name="cc_in")
                    bout2 = drp.tile([P, CK, TH], F16, name="cc_out")
                    for ct in range(CK):
                        w2_ct = w2p.tile([P, FCK, P], BF16, name="w2_ct")
                        nc.sync.dma_start(w2_ct[:], w2_d.ap()[l, ct])
                        fp = mmp.tile([P, TH], F32,
                                      name=("mA" if ct % 2 == 0 else "mB"))
                        for fc in range(FCK):
                            nc.tensor.matmul(
                                fp[:], w2_ct[:, fc], hT[:, fc],
                                start=(fc == 0), stop=(fc == FCK - 1))
                        fo_c = sap.tile([P, TH], F16, name="sa_c")
                        with nc.allow_low_precision(reason="f16 cc"):
                            nc.vector.tensor_scalar(
                                fo_c[:], fp[:], b2g[:, ct:ct + 1], None,
                                ALU.add)
                        nc.sync.dma_start(bin2[:, ct], fo_c[:])
                    nc.gpsimd.collective_compute(
                        "AllReduce", ALU.add, replica_groups=GROUPS,
                        ins=[bin2.opt()], outs=[bout2.opt()])
                    return bout2

                # ---------- two-stream pipelined layer body ----------
                # Every AllReduce is covered by >=14us of queued PE work:
                # AR_a0 by qkv_attn(1)+proj(1), AR_a1 by FFN(0), AR_f0 by
                # FFN(1), AR_f1 by the next layer's qkv_attn(0).
                qkv_attn(0)
                if l == 0:
                    tap("t_xn1", xnh[0][:, 0, :8])
                    tap("t_q0", qTa[0][0][:, :8])
                    tap("t_k0", kTa[0][0][:, :8])
                    tap("t_v0", vg[0][:, 0, :8])
                ar_a0 = proj_ar(0)
                qkv_attn(1)
                ar_a1 = proj_ar(1)
                resid(0, ar_a0, "t_sa" if l == 0 else None)
                if l == 0:
                    tap("t_x1", xTh[0][:, 0, :16].bitcast(F32))
                ar_f0 = ffn_ar(0)
                resid(1, ar_a1)
                ar_f1 = ffn_ar(1)
                resid(0, ar_f0, "t_ffn" if l == 0 else None)
                if l == 0:
                    tap("t_x2", xTh[0][:, 0, :16].bitcast(F32))
                resid(1, ar_f1)
            tap(f"t_xl{l}", xTh[0][:, 0, :16].bitcast(F32))

        # ======================= final LN + head =======================
        with tc.tile_pool(name="xf", bufs=1) as xfp:
            xfT = xfp.tile([P, CK, T], BF16)
            for th in range(2):
                ln_half(xfT, slice(th * TH, (th + 1) * TH), xTh[th])
            tap("t_xf", xfT[:, 0, :8])
            with (tc.tile_pool(name="hw", bufs=3) as hwp,
                  tc.tile_pool(name="lg", bufs=3) as lgp,
                  tc.tile_pool(name="hbp", bufs=1) as hbp):
                for vt in range(NVT):
                    vs = slice(vt * VT, (vt + 1) * VT)
                    hb_sb = hbp.tile([1, VT], F32R, name="hb_sb", bufs=2)
                    nc.sync.dma_start(hb_sb[:],
                                      hb_d.ap()[:, vs].bitcast(F32R))
                    hw_c = hwp.tile([P, CK, VT], BF16, name="hw_c")
                    nc.sync.dma_start(
                        hw_c[:],
                        hw_d.ap()[:, :, vs].rearrange("k p v -> p k v"))
                    bp = bcp.tile([P, VT], F32, name="bc")
                    nc.tensor.matmul(bp[:], ones_r[:1, :], hb_sb[:],
                                     start=True, stop=True)
                    bs = lgp.tile([P, VT], F32, name="bs")
                    nc.scalar.activation(bs[:], bp[:], AF.Copy)
                    for tt in range(T // P):
                        lp = mmp.tile([P, VT], F32,
                                      name=("mA" if tt % 2 == 0 else "mB"))
                        for k in range(CK):
                            nc.tensor.matmul(
                                lp[:], xfT[:, k, tt * P:(tt + 1) * P],
                                hw_c[:, k], start=(k == 0), stop=(k == CK - 1))
                        lg_sb = lgp.tile([P, VT], BF16, name="lg_sb")
                        with nc.allow_low_precision(reason="bf16 logits"):
                            nc.vector.tensor_add(lg_sb[:], lp[:], bs[:])
                        if vt == 0 and tt == 0:
                            tap("t_lg", lg_sb[:, :8])
                        nc.sync.dma_start(
                            logits_d.ap()[tt * P:(tt + 1) * P, vs], lg_sb[:])

    nc.compile()
    return nc


# ----------------------------------------------------------------------------
# host entry
# ----------------------------------------------------------------------------

def kernel(**inputs):
    from concourse.bass_utils import run_bass_kernel_spmd

    if "nc" not in _CACHE:
        _CACHE["nc"] = build_nc(taps=False)
    nc = _CACHE["nc"]

    # weights depend only on the TP half; share arrays across DP groups
    wd = [_prep_core(inputs, tp) for tp in range(2)]
    in_maps = []
    for c in range(8):
        b, tp = c // 2, c % 2
        m = dict(wd[tp])
        m["idx"] = np.ascontiguousarray(
            np.asarray(inputs["idx"][b]).astype(np.int32).reshape(T, 1))
        in_maps.append(m)
    res = run_bass_kernel_spmd(nc, in_maps, core_ids=list(range(8)))
    out = np.zeros((B, T, V), np.float32)
    for c in range(8):
        b, tp = c // 2, c % 2
        out[b, :, tp * VS:(tp + 1) * VS] = np.asarray(
            res.results[c]["logits"], dtype=np.float32)
    return out
